# revision 10
# baseline (speedup 1.0000x reference)
"""Spikformer block (Q/K/V linear+BN+{LIF,ReLU,ternary} -> headwise linear attention
-> O linear+BN+LIF) on 8 TRN2 NeuronCores, data-parallel over batch.

Self-contained: hardcodes shapes; builds one SPMD Bass program; shards batch
across 8 cores; gathers/transposes on host.

Key algebra:
  - attention has no softmax -> (q k^T) v reassociated as q (k^T v); per-head
    k^T v is 64x64, ~8x fewer MACs and no 512x512 attn matrix to evict.
  - all BatchNorms are eval-mode affine: folded into per-channel scale/bias on
    host; applied during PSUM eviction (per-partition APs) for the [e,t]-layout
    paths, or folded into weights + a bias matmul for the [t,e]-layout paths.
"""
import sys
for p in ("/opt/trn_rl_repo",):
    if p not in sys.path:
        sys.path.insert(0, p)

import numpy as np
import ml_dtypes

import concourse.bass as bass
import concourse.bacc as bacc
import concourse.mybir as mybir
import concourse.tile as tile
from concourse.bass_utils import run_bass_kernel_spmd

B, T, L, D, H = 8, 4, 512, 512, 8
HD = D // H
NT = T * L            # 2048 tokens per core
P = 128
EC = D // P           # 4 e-chunks
DC = D // P           # 4 d-chunks
TC = NT // P          # 16 token chunks (choice-2 layout)
EPS = 1e-5
F32 = mybir.dt.float32
BF16 = mybir.dt.bfloat16
F = mybir.ActivationFunctionType
ALU = mybir.AluOpType

LIN_MODE = "bf16x3"   # "fp32" (native, 4cyc/row) or "bf16x3" (split, 3cyc/row)

_PROGRAM_CACHE = {}
_last_in_maps = None


def _build_program():
    key = LIN_MODE
    if key in _PROGRAM_CACHE:
        return _PROGRAM_CACHE[key]

    nc = bacc.Bacc("TRN2", target_bir_lowering=False, debug=False, num_devices=8)

    dram = {}
    if LIN_MODE == "fp32":
        dram["xT"] = nc.dram_tensor("xT", [D, NT], F32, kind="ExternalInput")
        for w in ("wq", "wk", "wv", "wo"):
            dram[w] = nc.dram_tensor(w, [D, D], F32, kind="ExternalInput")
    else:
        for t_ in ("xTh", "xTl"):
            dram[t_] = nc.dram_tensor(t_, [D, NT], BF16, kind="ExternalInput")
        for w in ("wq", "wk", "wv", "wo"):
            for s in ("h", "l"):
                dram[w + s] = nc.dram_tensor(w + s, [D, D], BF16, kind="ExternalInput")
    for v_ in ("qs", "qb", "os_", "ob"):
        dram[v_] = nc.dram_tensor(v_, [D, 1], F32, kind="ExternalInput")
    dram["kb3"] = nc.dram_tensor("kb3", [3, D], BF16, kind="ExternalInput")
    dram["vb3"] = nc.dram_tensor("vb3", [3, D], BF16, kind="ExternalInput")
    dram["ones3"] = nc.dram_tensor("ones3", [3, P], BF16, kind="ExternalInput")
    out_d = nc.dram_tensor("out", [D, NT], F32, kind="ExternalOutput")

    with tile.TileContext(nc) as tc_:
        with tc_.tile_pool(name="sb", bufs=1) as sb, \
             tc_.tile_pool(name="sc", bufs=3) as sc, \
             tc_.tile_pool(name="sp8", bufs=6) as sp8, \
             tc_.tile_pool(name="ps", bufs=4, space="PSUM") as ps, \
             tc_.tile_pool(name="pk", bufs=2, space="PSUM") as pk:

            # ---------- persistent SBUF tiles ----------
            if LIN_MODE == "fp32":
                xT = [sb.tile([P, NT], F32, tag="xa", bufs=DC, name=f"xT{i}") for i in range(DC)]
                wts = {w: [sb.tile([P, D], F32, tag=w, bufs=DC, name=f"{w}{i}") for i in range(DC)]
                       for w in ("wq", "wk", "wv", "wo")}
            else:
                xTh = [sb.tile([P, NT], BF16, tag="xa", bufs=2 * DC, name=f"xTh{i}") for i in range(DC)]
                xTl = [sb.tile([P, NT], BF16, tag="xa", bufs=2 * DC, name=f"xTl{i}") for i in range(DC)]
                wts = {w + s: [sb.tile([P, D], BF16, tag=w + s, bufs=DC, name=f"{w}{s}{i}") for i in range(DC)]
                       for w in ("wq", "wk", "wv", "wo") for s in ("h", "l")}
            qT = [sb.tile([P, NT], BF16, tag="qT", bufs=EC, name=f"qT{i}") for i in range(EC)]       # q spikes [e,t]
            k_nat = [sb.tile([P, D], F32, tag="knat", bufs=TC, name=f"knat{i}") for i in range(TC)]   # k [t,e]
            v_nat = [sb.tile([P, D], F32, tag="vnat", bufs=TC, name=f"vnat{i}") for i in range(TC)]   # v [t,e]
            # attention output [e, t] as bf16 hi/lo pairs (exact enough for o-linear)
            if LIN_MODE == "fp32":
                ao = [sb.tile([P, NT], F32, tag="xa", bufs=DC, name=f"ao{i}") for i in range(DC)]
            else:
                aoh = [sb.tile([P, NT], BF16, tag="xa", bufs=2 * DC, name=f"aoh{i}") for i in range(DC)]
                aol = [sb.tile([P, NT], BF16, tag="xa", bufs=2 * DC, name=f"aol{i}") for i in range(DC)]
            memq = [sb.tile([P, L], F32, tag="memq", bufs=EC, name=f"memq{i}") for i in range(EC)]
            memo = [sb.tile([P, L], F32, tag="memo", bufs=EC, name=f"memo{i}") for i in range(EC)]
            consts = {v_: [sb.tile([P, 1], F32, tag="cst", bufs=4 * EC, name=f"c_{v_}{i}") for i in range(EC)]
                      for v_ in ("qs", "qb", "os_", "ob")}
            kb3 = sb.tile([3, D], BF16, tag="kb3")
            vb3 = sb.tile([3, D], BF16, tag="vb3")
            ones3 = sb.tile([3, P], BF16, tag="ones3")
            cneg1 = sb.tile([P, 1], F32, tag="cneg1")
            nc.gpsimd.memset(cneg1[:], -1.0)

            # ---------- loads ----------
            if LIN_MODE == "fp32":
                for dc in range(DC):
                    nc.sync.dma_start(xT[dc][:], dram["xT"][dc * P:(dc + 1) * P, :])
                for w in ("wq", "wk", "wv", "wo"):
                    for dc in range(DC):
                        nc.sync.dma_start(wts[w][dc][:], dram[w][dc * P:(dc + 1) * P, :])
            else:
                for dc in range(DC):
                    nc.sync.dma_start(xTh[dc][:], dram["xTh"][dc * P:(dc + 1) * P, :])
                    nc.sync.dma_start(xTl[dc][:], dram["xTl"][dc * P:(dc + 1) * P, :])
                for w in wts:
                    for dc in range(DC):
                        nc.sync.dma_start(wts[w][dc][:], dram[w][dc * P:(dc + 1) * P, :])
            for v_ in consts:
                for i in range(EC):
                    nc.sync.dma_start(consts[v_][i][:], dram[v_][i * P:(i + 1) * P, :])
            nc.sync.dma_start(kb3[:], dram["kb3"][:])
            nc.sync.dma_start(vb3[:], dram["vb3"][:])
            nc.sync.dma_start(ones3[:], dram["ones3"][:])

            def lin_mms(psum, w, lhs_xt=False, tc_=None, ec=None, ti=None, close=False):
                """Emit the matmul group for one linear output tile.
                close=True marks stop on the final matmul (no trailing bias mm)."""
                if LIN_MODE == "fp32":
                    for dc in range(DC):
                        if lhs_xt:   # choice-2: lhsT = xT chunk cols, rhs = weight
                            lhsT = xT[dc][:, tc_ * P:(tc_ + 1) * P]
                            rhs = wts[w][dc][:]
                        else:        # choice-1: lhsT = weight cols, rhs = xT cols
                            lhsT = wts[w][dc][:, ec * P:(ec + 1) * P]
                            rhs = xT[dc][:, ti * L:(ti + 1) * L]
                        nc.tensor.matmul(psum[:], lhsT, rhs,
                                         start=(dc == 0),
                                         stop=(close and dc == DC - 1))
                else:
                    i = 0
                    n = 3 * DC
                    for dc in range(DC):
                        for (xs, wsfx) in ((xTh, "h"), (xTh, "l"), (xTl, "h")):
                            i += 1
                            if lhs_xt:
                                lhsT = xs[dc][:, tc_ * P:(tc_ + 1) * P]
                                rhs = wts[w + wsfx][dc][:]
                            else:
                                lhsT = wts[w + wsfx][dc][:, ec * P:(ec + 1) * P]
                                rhs = xs[dc][:, ti * L:(ti + 1) * L]
                            nc.tensor.matmul(psum[:], lhsT, rhs,
                                             start=(i == 1),
                                             stop=(close and i == n))

            # ---------- phase 1: Q linear + LIF (choice-1: [e, t]) ----------
            for ti in range(T):
                for ec in range(EC):
                    pq = ps.tile([P, L], F32, tag="mm512")
                    lin_mms(pq, "wq", ec=ec, ti=ti, close=True)
                    u = sc.tile([P, L], F32, tag="u")
                    if ti == 0:
                        nc.vector.tensor_scalar(memq[ec][:], pq[:], consts["qs"][ec][:],
                                                consts["qb"][ec][:], ALU.mult, ALU.add)
                    else:
                        nc.vector.tensor_scalar(u[:], pq[:], consts["qs"][ec][:],
                                                consts["qb"][ec][:], ALU.mult, ALU.add)
                        nc.vector.scalar_tensor_tensor(memq[ec][:], memq[ec][:], 0.5, u[:],
                                                       ALU.mult, ALU.add)
                        nc.gpsimd.tensor_sub(memq[ec][:], memq[ec][:],
                                             qT[ec][:, (ti - 1) * L:ti * L])
                    nc.vector.tensor_scalar(qT[ec][:, ti * L:(ti + 1) * L],
                                            memq[ec][:], 1.0, None, ALU.is_ge)

            # ---------- phase 1b: K and V linears (choice-2: [t, e]) ----------
            for tc2 in range(TC):
                pkv = ps.tile([P, D], F32, tag="mm512")
                lin_mms(pkv, "wk", lhs_xt=True, tc_=tc2)
                nc.tensor.matmul(pkv[:], ones3[:], kb3[:], start=False, stop=True)
                nc.scalar.activation(k_nat[tc2][:], pkv[:], F.Relu)

                pvv = ps.tile([P, D], F32, tag="mm512")
                lin_mms(pvv, "wv", lhs_xt=True, tc_=tc2)
                nc.tensor.matmul(pvv[:], ones3[:], vb3[:], start=False, stop=True)
                t1 = sc.tile([P, D], F32, tag="t1")
                t2 = sc.tile([P, D], F32, tag="t2")
                nc.scalar.activation(t1[:], pvv[:], F.Sign, bias=cneg1[:], scale=1.0)
                nc.scalar.activation(t2[:], pvv[:], F.Sign, bias=1.0, scale=1.0)
                nc.gpsimd.tensor_add(v_nat[tc2][:], t1[:], t2[:])

            # ---------- phase 2: attention  out^T[e,l] = kv^T q^T ----------
            for ti in range(T):
                for c in range(EC):          # head pair (2c, 2c+1) == e-chunk c
                    pkv64 = pk.tile([P, HD], F32, tag="kv64")
                    for hh in range(2):
                        h = 2 * c + hh
                        off = hh * HD
                        for mc in range(4):
                            tc2 = ti * 4 + mc
                            nc.tensor.matmul(
                                pkv64[off:off + HD, :],
                                k_nat[tc2][:, h * HD:(h + 1) * HD],
                                v_nat[tc2][:, h * HD:(h + 1) * HD],
                                start=(mc == 0), stop=(mc == 3),
                                tile_position=(0, off))
                    kv1 = sc.tile([P, HD], BF16, tag="kv1")
                    kvd = sc.tile([P, HD], F32, tag="kvd")
                    kv2 = sc.tile([P, HD], BF16, tag="kv2")
                    kv3 = sc.tile([P, HD], BF16, tag="kv3")
                    nc.scalar.copy(kv1[:], pkv64[:])
                    nc.vector.tensor_tensor(kvd[:], pkv64[:], kv1[:], ALU.subtract)
                    nc.vector.tensor_copy(kv2[:], kvd[:])
                    nc.vector.tensor_tensor(kv3[:], kvd[:], kv2[:], ALU.subtract)
                    pso = ps.tile([P, L], F32, tag="mm512")
                    for off in (0, HD):
                        for j, kvt in enumerate((kv1, kv2, kv3)):
                            nc.tensor.matmul(pso[off:off + HD, :], kvt[off:off + HD, :],
                                             qT[c][off:off + HD, ti * L:(ti + 1) * L],
                                             start=(j == 0), stop=(j == 2),
                                             tile_position=(off, off))
                    if LIN_MODE == "fp32":
                        nc.scalar.copy(ao[c][:, ti * L:(ti + 1) * L], pso[:])
                    else:
                        nc.scalar.copy(aoh[c][:, ti * L:(ti + 1) * L], pso[:])
                        nc.vector.tensor_tensor(aol[c][:, ti * L:(ti + 1) * L],
                                                pso[:], aoh[c][:, ti * L:(ti + 1) * L],
                                                ALU.subtract)

            # ---------- phase 3: O linear + BN + LIF ----------
            prev_spk = [None] * EC
            for ti in range(T):
                for ec in range(EC):
                    po = ps.tile([P, L], F32, tag="mm512")
                    if LIN_MODE == "fp32":
                        for dc in range(DC):
                            nc.tensor.matmul(po[:], wts["wo"][dc][:, ec * P:(ec + 1) * P],
                                             ao[dc][:, ti * L:(ti + 1) * L],
                                             start=(dc == 0), stop=(dc == DC - 1))
                    else:
                        first = True
                        n3 = 3 * DC
                        i = 0
                        for dc in range(DC):
                            for (asrc, wsfx) in ((aoh, "h"), (aoh, "l"), (aol, "h")):
                                i += 1
                                nc.tensor.matmul(po[:], wts["wo" + wsfx][dc][:, ec * P:(ec + 1) * P],
                                                 asrc[dc][:, ti * L:(ti + 1) * L],
                                                 start=first, stop=(i == n3))
                                first = False
                    u = sc.tile([P, L], F32, tag="u")
                    if ti == 0:
                        nc.vector.tensor_scalar(memo[ec][:], po[:], consts["os_"][ec][:],
                                                consts["ob"][ec][:], ALU.mult, ALU.add)
                    else:
                        nc.vector.tensor_scalar(u[:], po[:], consts["os_"][ec][:],
                                                consts["ob"][ec][:], ALU.mult, ALU.add)
                        nc.vector.scalar_tensor_tensor(memo[ec][:], memo[ec][:], 0.5, u[:],
                                                       ALU.mult, ALU.add)
                        nc.gpsimd.tensor_sub(memo[ec][:], memo[ec][:], prev_spk[ec][:])
                    spk = sp8.tile([P, L], F32, tag="ospk")
                    nc.vector.tensor_scalar(spk[:], memo[ec][:], 1.0, None, ALU.is_ge)
                    prev_spk[ec] = spk
                    nc.sync.dma_start(out_d[ec * P:(ec + 1) * P, ti * L:(ti + 1) * L], spk[:])

    nc.compile()
    _PROGRAM_CACHE[key] = nc
    return nc


def _split_bf16(a):
    hi = a.astype(ml_dtypes.bfloat16)
    lo = (a - hi.astype(np.float32)).astype(ml_dtypes.bfloat16)
    return hi, lo


def _split3_bf16(a):
    h1 = a.astype(ml_dtypes.bfloat16)
    r = a - h1.astype(np.float32)
    h2 = r.astype(ml_dtypes.bfloat16)
    h3 = (r - h2.astype(np.float32)).astype(ml_dtypes.bfloat16)
    return np.stack([h1, h2, h3])


def kernel(**inputs):
    nc = _build_program()

    f64 = np.float64
    x = np.asarray(inputs["x"], np.float32)

    def bn_fold(g, b_, rm, rv):
        s = (g.astype(f64) / np.sqrt(rv.astype(f64) + EPS))
        bias = b_.astype(f64) - rm.astype(f64) * s
        return s, bias

    sq, bq = bn_fold(inputs["q_g"], inputs["q_b"], inputs["q_rm"], inputs["q_rv"])
    sk, bk = bn_fold(inputs["k_g"], inputs["k_b"], inputs["k_rm"], inputs["k_rv"])
    sv, bv = bn_fold(inputs["v_g"], inputs["v_b"], inputs["v_rm"], inputs["v_rv"])
    so, bo = bn_fold(inputs["o_g"], inputs["o_b"], inputs["o_rm"], inputs["o_rv"])
    C = 0.5 * HD ** -0.5   # extra 0.5 compensates ternary v stored as {-2,0,2}
    # o path: out = bn(lin + o_bias) -> bias' = (o_bias - rm)*s + b
    bo = bo + inputs["o_bias"].astype(f64) * so

    wq = inputs["q_w"].astype(f64)
    wk = inputs["k_w"].astype(f64) * (C * sk)[:, None]
    wv = inputs["v_w"].astype(f64) * sv[:, None]
    wo = inputs["o_w"].astype(f64)
    kb_fold = (C * bk).astype(np.float32)
    vb_fold = bv.astype(np.float32)

    shared = {
        "qs": sq.astype(np.float32).reshape(D, 1),
        "qb": bq.astype(np.float32).reshape(D, 1),
        "os_": so.astype(np.float32).reshape(D, 1),
        "ob": bo.astype(np.float32).reshape(D, 1),
        "kb3": _split3_bf16(kb_fold),
        "vb3": _split3_bf16(vb_fold),
        "ones3": np.ones((3, P), dtype=ml_dtypes.bfloat16),
    }
    if LIN_MODE == "fp32":
        shared["wq"] = np.ascontiguousarray(wq.astype(np.float32).T)
        shared["wk"] = np.ascontiguousarray(wk.astype(np.float32).T)
        shared["wv"] = np.ascontiguousarray(wv.astype(np.float32).T)
        shared["wo"] = np.ascontiguousarray(wo.astype(np.float32).T)
    else:
        for name, w in (("wq", wq), ("wk", wk), ("wv", wv), ("wo", wo)):
            hi, lo = _split_bf16(np.ascontiguousarray(w.astype(np.float32).T))
            shared[name + "h"] = hi
            shared[name + "l"] = lo

    in_maps = []
    for b in range(B):
        xT = np.ascontiguousarray(x[b].reshape(NT, D).T)   # (D, NT)
        m = dict(shared)
        if LIN_MODE == "fp32":
            m["xT"] = xT
        else:
            hi, lo = _split_bf16(xT)
            m["xTh"] = hi
            m["xTl"] = lo
        in_maps.append(m)

    global _last_in_maps
    _last_in_maps = in_maps
    res = run_bass_kernel_spmd(nc, in_maps, core_ids=list(range(B)))
    outs = []
    for b in range(B):
        oT = res.results[b]["out"]                    # (D, NT)
        outs.append(oT.reshape(D, T, L).transpose(1, 2, 0))
    return np.stack(outs).astype(np.float32)


if __name__ == "__main__":
    import importlib.util
    spec = importlib.util.spec_from_file_location("reference", "/root/problem/reference.py")
    ref = importlib.util.module_from_spec(spec)
    spec.loader.exec_module(ref)
    inp = {k: np.asarray(v) for k, v in ref.setup_inputs().items()}
    exp = np.asarray(ref.reference(**inp))
    act = kernel(**inp)
    rel = np.linalg.norm(act - exp) / np.linalg.norm(exp)
    print("flips:", int(np.sum(act != exp)), "/", exp.size)
    print("Relative error:", rel)
